# revision 48
# baseline (speedup 1.0000x reference)
"""CrossAttention kernel for 8 Trainium2 NeuronCores.

Problem (hardcoded shapes): B=4, N=1024, C=1024, E=1024, H=16, D=64.
  kv = x @ Wkv + bkv ; k, v = split(kv) ; q = query @ Wq + bq
  keys = [k; q] (2N), values = [v; v]
  out = softmax(q keys^T / sqrt(D)) @ values        -> [B, N, E]

Sharding: 8 cores = 4 batches x 2 head-groups (8 heads each).

Design (cost-model-driven):
  - ACT-engine exp stream is the critical resource (~133us of exp busy).
    One "unit" = (head, key-tile): scores^T [k 128, q 1024] in PSUM ->
    exp -> SBUF bf16. 128 units emitted back-to-back keep ACT saturated.
  - All attention matmuls in bf16 (cost model: 1.0 cycles/row at any
    moving size). PV runs in natural layout: stationary = exp-tile
    column block [k 128, q 128], moving = v|1 [k 128, 65] -> out
    [q 128, 65] accumulated in PSUM over the 16 key tiles (the 65th
    column is the softmax denominator).
  - PV accumulators for the 8 q-tiles of one head pack into 2 PSUM
    banks (qt0-6 at 65-f32 stride in bank A, qt7 in bank B). PSUM
    "zero regions" are 2KB: only the first matmul written into each
    bank carries start=True (lazy-zero covers the other groups) and
    only the last written carries stop=True.
  - Projections (Q/K/V, f32->bf16 inputs host-cast) are emitted as
    filler inside the unit stream so the PE never idles and the first
    exp starts as soon as qT(tile0) is projected (q-block keys first).
  - PSUM: 2x scores [128,1024] (4 banks) + pv (2) + 2x proj (2) = 8.
"""
import numpy as np

B, N, C, E, H = 4, 1024, 1024, 1024, 16
D = E // H            # 64
HPC = 8               # heads per core
EC = HPC * D          # 512 E-columns per core
NCORES = 8
CT = C // 128         # 8 contraction tiles
ST = N // 128         # 8 seq tiles
KT = 2 * N // 128     # 16 key tiles (k then q-as-keys)
# head 0 runs all q-block keys first (kT isn't projected yet); later heads
# interleave [q-key, k-key] pairs so the exp-pair adds and PV groups spread
# evenly across the head's window.
KTORDER0 = list(range(8, 16)) + list(range(8))
KTORDERI = [x for j in range(8) for x in (8 + j, j)]

_compiled = None


def _build():
    import concourse.bass as bass
    import concourse.bacc as bacc
    import concourse.mybir as mybir
    import concourse.tile as tile
    import contextlib

    F32 = mybir.dt.float32
    BF16 = mybir.dt.bfloat16
    EXP = mybir.ActivationFunctionType.Exp
    from concourse.alu_op_type import AluOpType
    MUL = AluOpType.mult
    ADD = AluOpType.add

    nc = bacc.Bacc()
    xta_in = nc.declare_dram_parameter("xta", [C, 512], BF16, isOutput=False)
    xtb_in = nc.declare_dram_parameter("xtb", [C, 512], BF16, isOutput=False)
    qta_in = nc.declare_dram_parameter("qta", [C, 512], BF16, isOutput=False)
    qtb_in = nc.declare_dram_parameter("qtb", [C, 512], BF16, isOutput=False)
    wqa_in = nc.declare_dram_parameter("wqa", [C, 128], BF16, isOutput=False)
    wqb_in = nc.declare_dram_parameter("wqb", [C, 384], BF16, isOutput=False)
    wka_in = nc.declare_dram_parameter("wka", [C, 128], BF16, isOutput=False)
    wkb_in = nc.declare_dram_parameter("wkb", [C, 384], BF16, isOutput=False)
    wv_in = nc.declare_dram_parameter("wv", [C, EC], BF16, isOutput=False)
    bq_in = nc.declare_dram_parameter("bq", [EC], F32, isOutput=False)
    bk_in = nc.declare_dram_parameter("bk", [EC], F32, isOutput=False)
    bv_in = nc.declare_dram_parameter("bv", [EC], F32, isOutput=False)
    out_o = nc.declare_dram_parameter("out_t", [N, EC], F32, isOutput=True)

    with tile.TileContext(nc) as tc, contextlib.ExitStack() as ctx:
        pers = ctx.enter_context(tc.tile_pool(name="pers", bufs=1))
        epool = ctx.enter_context(tc.tile_pool(name="epool", bufs=24))
        wpool = ctx.enter_context(tc.tile_pool(name="wpool", bufs=30))
        opool = ctx.enter_context(tc.tile_pool(name="opool", bufs=2))
        rcpool = ctx.enter_context(tc.tile_pool(name="rcpool", bufs=4))
        scp = ctx.enter_context(tc.tile_pool(name="scp", bufs=2, space="PSUM"))
        psp = ctx.enter_context(tc.tile_pool(name="psp", bufs=1, space="PSUM"))
        pjp = ctx.enter_context(tc.tile_pool(name="pjp", bufs=2, space="PSUM"))

        # ---- persistent SBUF ----
        xTs = pers.tile([128, CT, N], BF16, tag="xTs", name="xTs")
        qryTs = pers.tile([128, CT, N], BF16, tag="qryTs", name="qryTs")
        wqas = pers.tile([128, CT, 128], BF16, tag="wqas", name="wqas")
        wqbs = pers.tile([128, CT, 384], BF16, tag="wqbs", name="wqbs")
        wkas = pers.tile([128, CT, 128], BF16, tag="wkas", name="wkas")
        wkbs = pers.tile([128, CT, 384], BF16, tag="wkbs", name="wkbs")
        wvs = pers.tile([128, CT, EC], BF16, tag="wvs", name="wvs")
        kTs = pers.tile([128, 4, N], BF16, tag="kTs", name="kTs")
        qTs = pers.tile([128, 4, N], BF16, tag="qTs", name="qTs")
        vvs = pers.tile([128, ST, HPC, D + 1], BF16, tag="vvs", name="vvs")
        bqc = pers.tile([128, 4], F32, tag="bqc", name="bqc")
        bkc = pers.tile([128, 4], F32, tag="bkc", name="bkc")
        bvb = pers.tile([128, EC], F32, tag="bvb", name="bvb")

        # persistent PSUM: per-head PV accumulators, banks A|B
        # qt<7 at element offset 65*qt (bank A), qt7 at 512 (bank B)
        pvacc = psp.tile([128, 1024], F32, tag="pvacc", name="pvacc")

        def pvoff(qt):
            return 65 * qt if qt < 7 else 512

        # ---- constants (no DMA) ----
        nc.gpsimd.memset(vvs[:, :, :, D:D + 1], 1.0)

        # ---- input DMAs in consumption-priority order ----
        nc.sync.dma_start(out=bqc[:], in_=bq_in.ap().rearrange("(t p) -> p t", p=128))
        nc.sync.dma_start(out=bkc[:], in_=bk_in.ap().rearrange("(t p) -> p t", p=128))
        _b = bv_in.ap()
        bv_bc = bass.AP(tensor=_b.tensor, offset=_b.offset, ap=[[0, 128], [1, EC]])
        nc.sync.dma_start(out=bvb[:], in_=bv_bc)
        for ct in range(CT):
            nc.sync.dma_start(out=wqas[:, ct, :], in_=wqa_in[ct * 128:(ct + 1) * 128, :])
        for ct in range(CT):
            nc.sync.dma_start(out=qryTs[:, ct, 0:512], in_=qta_in[ct * 128:(ct + 1) * 128, :])
        for ct in range(CT):
            nc.sync.dma_start(out=qryTs[:, ct, 512:1024], in_=qtb_in[ct * 128:(ct + 1) * 128, :])
        for ct in range(CT):
            nc.sync.dma_start(out=wkas[:, ct, :], in_=wka_in[ct * 128:(ct + 1) * 128, :])
        for ct in range(CT):
            nc.sync.dma_start(out=xTs[:, ct, 0:512], in_=xta_in[ct * 128:(ct + 1) * 128, :])
        for ct in range(CT):
            nc.sync.dma_start(out=xTs[:, ct, 512:1024], in_=xtb_in[ct * 128:(ct + 1) * 128, :])
        for ct in range(CT):
            nc.sync.dma_start(out=wkbs[:, ct, :], in_=wkb_in[ct * 128:(ct + 1) * 128, :])
        for ct in range(CT):
            nc.sync.dma_start(out=wqbs[:, ct, :], in_=wqb_in[ct * 128:(ct + 1) * 128, :])
        for ct in range(CT):
            nc.sync.dma_start(out=wvs[:, ct, :], in_=wv_in[ct * 128:(ct + 1) * 128, :])

        # ---- projection work items --------------------------------------
        # Each item: 8 ct-matmuls into one [128,512] PSUM bank + 1 epilogue.
        # kind: ("Q"|"K", tile, half) -> kTs/qTs ; ("V", st) -> vvs
        def emit_proj_mm(kind, a, b, ct, pj):
            if kind == "V":
                nc.tensor.matmul(pj[:], xTs[:, ct, a * 128:(a + 1) * 128],
                                 wvs[:, ct, :], start=(ct == 0), stop=(ct == CT - 1))
            else:
                ws_a, ws_b = (wqas, wqbs) if kind == "Q" else (wkas, wkbs)
                stat = (ws_a[:, ct, :] if a == 0
                        else ws_b[:, ct, (a - 1) * 128:a * 128])
                src = qryTs if kind == "Q" else xTs
                nc.tensor.matmul(pj[:], stat, src[:, ct, b * 512:(b + 1) * 512],
                                 start=(ct == 0), stop=(ct == CT - 1))

        def emit_proj_epi(kind, a, b, pj):
            if kind == "V":
                nc.vector.tensor_copy(
                    out=vvs[:, a, :, 0:D],
                    in_=pj[:].rearrange("p (h d) -> p h d", h=HPC))
            else:
                dst, bc = (qTs, bqc) if kind == "Q" else (kTs, bkc)
                nc.vector.tensor_scalar_add(
                    out=dst[:, a, b * 512:(b + 1) * 512], in0=pj[:],
                    scalar1=bc[:, a:a + 1])

        # Projection micro-op stream: each element is a closure emitting one
        # matmul (~213ns) or an epilogue (DVE, free for PE budget). K/Q
        # tiles emit per seq-half (each gated on its own input DMA).
        vepi_done = [False] * ST
        ktile_done = [[False, False] for _ in range(4)]
        qtile_done = [[t == 0, t == 0] for t in range(4)]
        proj_ops = []  # list of (cost_ns, fn)

        def add_kq_half(kind, t, b):
            pj = [None]

            def mk_mm(ct):
                def f():
                    if ct == 0:
                        pj[0] = pjp.tile([128, 512], F32, tag="pj", name="pj")
                    emit_proj_mm(kind, t, b, ct, pj[0])
                    if ct == CT - 1:
                        emit_proj_epi(kind, t, b, pj[0])
                        (ktile_done if kind == "K" else qtile_done)[t][b] = True
                return f

            for ct in range(CT):
                proj_ops.append((225, mk_mm(ct)))

        def add_v_tile(st):
            pj = [None]

            def mk_mm(ct):
                def f():
                    if ct == 0:
                        pj[0] = pjp.tile([128, 512], F32, tag="pj", name="pj")
                    emit_proj_mm("V", st, 0, ct, pj[0])
                    if ct == CT - 1:
                        emit_proj_epi("V", st, 0, pj[0])
                        vepi_done[st] = True
                return f

            for ct in range(CT):
                proj_ops.append((213, mk_mm(ct)))

        # each op: (cost_ns, min_unit, fn). min_unit gates emission so PE
        # never head-of-line blocks on an input DMA that lands late.
        add_kq_half("K", 0, 0)   # xta lands ~unit 6-10
        add_kq_half("K", 0, 1)   # xtb lands ~unit 11-14
        add_kq_half("K", 1, 0)   # wkb lands ~unit 16
        add_kq_half("K", 1, 1)
        add_kq_half("Q", 1, 0)   # wqb lands ~unit 18
        add_kq_half("Q", 1, 1)
        add_v_tile(0)            # wv lands ~unit 21
        add_v_tile(1)
        add_v_tile(2)
        add_v_tile(3)
        add_kq_half("K", 2, 0)
        add_kq_half("K", 2, 1)
        add_kq_half("Q", 2, 0)
        add_kq_half("Q", 2, 1)
        add_v_tile(4)
        add_v_tile(5)
        add_v_tile(6)
        add_v_tile(7)
        add_kq_half("K", 3, 0)
        add_kq_half("K", 3, 1)
        add_kq_half("Q", 3, 0)
        add_kq_half("Q", 3, 1)

        gates = []
        gates += [6 + ct // 2 for ct in range(CT)]   # K0 half0
        gates += [11 + ct // 3 for ct in range(CT)]  # K0 half1
        gates += [16] * 16                           # K1
        gates += [18] * 16                           # Q1
        gates += [21, 21, 22, 22][0:1] * 0 + sum(([21 + st // 2] * CT for st in range(4)), [])  # V0..V3
        gates += [23] * 32                           # K2,Q2
        gates += [23] * (4 * CT)                     # V4..V7
        gates += [24] * 32                           # K3,Q3
        assert len(gates) == len(proj_ops), (len(gates), len(proj_ops))
        proj_ops = [(c, g, f) for (c, f), g in zip(proj_ops, gates)]
        proj_ops.reverse()  # pop from the end

        def emit_proj_budget(u, ns):
            spent = 0
            while proj_ops and proj_ops[-1][1] <= u \
                    and spent + proj_ops[-1][0] <= ns:
                cost, _, fn = proj_ops.pop()
                fn()
                spent += cost
            return spent

        # ---- lead-in: project qT tile 0 (heads 0,1) ----------------------
        for b_ in range(2):
            pj = pjp.tile([128, 512], F32, tag="pj", name="pj")
            for ct in range(CT):
                emit_proj_mm("Q", 0, b_, ct, pj)
            emit_proj_epi("Q", 0, b_, pj)

        # ---- attention unit stream ---------------------------------------
        # values are duplicated ([v; v]), so the kt and kt+8 exp tiles can
        # be pre-combined (one DVE add) and PV accumulates over 8 tiles
        # instead of 16 -- halves PV matmul count.
        pv_pending = []          # FIFO of (h, st)
        pv_count = [0] * HPC     # groups emitted per head
        etile = {}               # (h, kt) -> exp sbuf tile (q-keys half)
        normed = 0

        def pv_ready(h, st):
            return vepi_done[st]

        def emit_pv_group(h, st):
            w = etile.pop((h, st))
            first = pv_count[h] == 0
            last = pv_count[h] == ST - 1
            for qt in range(8):
                off = pvoff(qt)
                # start only on the first write into each 2KB zero region
                # (qt0 -> bank A, qt7 -> bank B); stop on the last write
                # (qt6 -> bank A, qt7 -> bank B).
                nc.tensor.matmul(pvacc[:, off:off + D + 1],
                                 w[:, qt * 128:(qt + 1) * 128],
                                 vvs[:, st, h, :],
                                 start=first and qt in (0, 7),
                                 stop=last and qt in (6, 7),
                                 skip_group_check=True)
            pv_count[h] += 1

        # output staging: one [128, 8, 2*64] tile per head PAIR; a single
        # wide DMA (512B lines) ships both heads' columns for all q-tiles.
        osb_cur = [None]

        def emit_norm(h):
            rc = rcpool.tile([128, 8], F32, tag="rc", name="rc")
            # denominators: qt0-6 at 65*qt+64 (stride 65), qt7 at 512+64
            dref = pvacc[:, D:D + 1]
            dn_strided = bass.AP(tensor=dref.tensor, offset=dref.offset,
                                 ap=[list(dref.ap[0]), [65, 7], [1, 1]])
            nc.vector.reciprocal(out=rc[:, 0:7], in_=dn_strided)
            nc.vector.reciprocal(out=rc[:, 7:8], in_=pvacc[:, 512 + D:512 + D + 1])
            if h % 2 == 0:
                osb_cur[0] = opool.tile([128, 8, 2 * D], F32, tag="osb", name="osb")
            osb = osb_cur[0]
            j = h % 2
            for qt in range(8):
                off = pvoff(qt)
                nc.vector.scalar_tensor_tensor(
                    out=osb[:, qt, j * D:(j + 1) * D], in0=pvacc[:, off:off + D],
                    scalar=rc[:, qt:qt + 1],
                    in1=bvb[:, h * D:(h + 1) * D], op0=MUL, op1=ADD)
            if h % 2 == 1:
                o_ap = out_o[0:128, (h - 1) * D:(h + 1) * D]
                o_wide = bass.AP(tensor=o_ap.tensor, offset=o_ap.offset,
                                 ap=[[EC, 128], [128 * EC, 8], [1, 2 * D]])
                nc.sync.dma_start(out=o_wide, in_=osb[:])

        # Unit list: head 0's q-key scores run at half-q granularity (the
        # exp stream starts as soon as qta+wqa have landed); all later
        # units are full [k 128, q 1024] tiles with interleaved kt order.
        UNITS = []
        UNITS += [(0, 8 + j, 0) for j in range(4)]
        UNITS += [(0, 8 + j, 1) for j in range(4)]
        UNITS += [(0, 12 + j, 0) for j in range(4)]
        UNITS += [(0, 12 + j, 1) for j in range(4)]
        UNITS += [(0, j, None) for j in range(8)]
        for h in range(1, HPC):
            UNITS += [(h, kt, None) for kt in KTORDERI]
        NUNITS = len(UNITS)

        def emit_unit(u):
            h, kt, hb = UNITS[u]
            t = h // 2
            src = kTs if kt < ST else qTs
            ksl = slice((kt % ST) * 128, (kt % ST) * 128 + 128)
            rows = slice((h % 2) * 64, (h % 2) * 64 + 64)
            if hb is not None:
                # half-q scores for (h, kt): [k-tile 128, q 512]
                assert qtile_done[t][(kt - 8) // 4] and qtile_done[t][hb], u
                s2 = scp.tile([128, N], F32, tag="sc", name="s2")
                nc.tensor.matmul(s2[:, 0:512], src[rows, t, ksl],
                                 qTs[rows, t, hb * 512:(hb + 1) * 512])
                if hb == 0:
                    e = epool.tile([128, N], BF16, tag="e", name="e")
                    etile[(h, kt - ST)] = e
                else:
                    e = etile[(h, kt - ST)]
                nc.scalar.activation(out=e[:, hb * 512:(hb + 1) * 512],
                                     in_=s2[:, 0:512], func=EXP, scale=0.125)
                return
            # full scores^T for (h, kt): [k-tile 128, q 1024]
            assert all(qtile_done[t]), (u, h, kt)
            if kt < ST:
                assert ktile_done[t][kt // 4], (u, h, kt)
            else:
                assert qtile_done[t][(kt - 8) // 4], (u, h, kt)
            s2 = scp.tile([128, N], F32, tag="sc", name="s2")
            for qb in range(2):
                nc.tensor.matmul(s2[:, qb * 512:(qb + 1) * 512],
                                 src[rows, t, ksl],
                                 qTs[rows, t, qb * 512:(qb + 1) * 512])
            e = epool.tile([128, N], BF16, tag="e", name="e")
            nc.scalar.activation(out=e[:], in_=s2[:], func=EXP, scale=0.125)
            if kt >= ST:
                etile[(h, kt - ST)] = e          # q-keys half, kept for add
            else:
                w = wpool.tile([128, N], BF16, tag="w", name="w")
                nc.vector.tensor_add(out=w[:], in0=e[:], in1=etile[(h, kt)])
                etile[(h, kt)] = w
                pv_pending.append((h, kt))       # (h, st): w ready

        emit_unit(0)
        for u in range(NUNITS):
            # scores for the NEXT unit go first so the PE always has the
            # critical instruction ahead of elastic filler work.
            if u + 1 < NUNITS:
                emit_unit(u + 1)
            # ---- PE filler: proj+PV work per unit (calibrated costs:
            # proj matmul ~225ns, PV group ~312ns incl per-instr overhead)
            if u < 6:
                budget = 230
            elif u < 16:
                budget = 460
            elif u < NUNITS - 32:
                budget = 680
            else:
                budget = 700
            npv = 2 if u >= NUNITS - 32 else 1
            while npv and pv_pending and pv_ready(*pv_pending[0]) \
                    and budget >= 312:
                hh, kk = pv_pending.pop(0)
                emit_pv_group(hh, kk)
                budget -= 312
                npv -= 1
                if pv_count[hh] == ST:
                    emit_norm(hh)
                    normed += 1
            emit_proj_budget(u, budget)

        # drain remaining proj (none expected) and PV backlog
        while proj_ops:
            cost, _, fn = proj_ops.pop()
            fn()
        while pv_pending:
            hh, kk = pv_pending.pop(0)
            if not pv_ready(hh, kk):
                raise RuntimeError("V projection missing at drain time")
            emit_pv_group(hh, kk)
            if pv_count[hh] == ST:
                emit_norm(hh)
                normed += 1
        assert normed == HPC, normed

    nc.finalize()
    return nc


def _get_compiled():
    global _compiled
    if _compiled is None:
        _compiled = _build()
    return _compiled


def kernel(x, query, Wkv, bkv, Wq, bq):
    import ml_dtypes
    from concourse.bass_utils import run_bass_kernel_spmd

    bf16 = ml_dtypes.bfloat16
    x = np.asarray(x, dtype=np.float32)
    query = np.asarray(query, dtype=np.float32)
    Wkv = np.asarray(Wkv, dtype=np.float32)
    bkv = np.asarray(bkv, dtype=np.float32)
    Wq = np.asarray(Wq, dtype=np.float32)
    bq = np.asarray(bq, dtype=np.float32)

    xT = [np.ascontiguousarray(x[b].T).astype(bf16) for b in range(B)]
    qT = [np.ascontiguousarray(query[b].T).astype(bf16) for b in range(B)]
    in_maps = []
    for core in range(NCORES):
        b, hg = core // 2, core % 2
        ecs = slice(hg * EC, (hg + 1) * EC)
        wq_c = Wq[:, ecs].astype(bf16)
        wk_c = Wkv[:, hg * EC:(hg + 1) * EC].astype(bf16)
        in_maps.append({
            "xta": np.ascontiguousarray(xT[b][:, 0:512]),
            "xtb": np.ascontiguousarray(xT[b][:, 512:1024]),
            "qta": np.ascontiguousarray(qT[b][:, 0:512]),
            "qtb": np.ascontiguousarray(qT[b][:, 512:1024]),
            "wqa": np.ascontiguousarray(wq_c[:, 0:128]),
            "wqb": np.ascontiguousarray(wq_c[:, 128:512]),
            "wka": np.ascontiguousarray(wk_c[:, 0:128]),
            "wkb": np.ascontiguousarray(wk_c[:, 128:512]),
            "wv": Wkv[:, E + hg * EC:E + (hg + 1) * EC].astype(bf16),
            "bq": np.ascontiguousarray(bq[ecs]),
            "bk": np.ascontiguousarray(bkv[hg * EC:(hg + 1) * EC]),
            "bv": np.ascontiguousarray(bkv[E + hg * EC:E + (hg + 1) * EC]),
        })

    nc = _get_compiled()
    res = None
    last_err = None
    for attempt in range(3):
        try:
            res = run_bass_kernel_spmd(nc, in_maps, list(range(NCORES)))
            break
        except Exception as ex:  # transient NRT_EXEC_UNIT_UNRECOVERABLE etc.
            last_err = ex
    if res is None:
        raise last_err

    out = np.empty((B, N, E), np.float32)
    for core in range(NCORES):
        b, hg = core // 2, core % 2
        out[b, :, hg * EC:(hg + 1) * EC] = res.results[core]["out_t"]
    return out
